# revision 6
# baseline (speedup 1.0000x reference)
"""Trainium2 Bass kernel for nn_InvariantResidualInteraction (8-core SPMD).

Strategy (graph/data parallel, per sharding hint):
  - Partition nodes across the 8 cores; each edge is owned by its receiver's
    core. Host-side "halo exchange": sender features are gathered per edge at
    sharding time and uploaded in the layout the kernel wants.
  - Nodes are bin-packed by in-degree into groups of <=16 nodes whose total
    in-degree is <=128, so each group's messages reduce with one K=128
    scatter-matmul (selection matrix built on-device from receiver indices).
  - Weights are replicated; scalar normalizations are folded into the weights.

Device pipeline per super-tile (8 groups = 1024 edge slots = 128 nodes):
  xs = F_gathered @ W_up            (TensorE, per 128-slot tile)
  S[e, n]   = (iota_n == recv_local[e])            (DVE iota+is_equal)
  SY[e, lm, n] = Y[e, lm] * S[e, n]                (DVE broadcast mult)
  z[e, l, c]   = R[e, l, c] * xs[e, c]             (DVE broadcast mult)
  A[c, lm, n]  = sum_e z[e, l(lm), c] * SY[e, lm, n]   (TensorE, 4 matmuls/group)
  out[k, lm, n] = sum_c W_l[c, k] * A[c, lm, n]        (TensorE, 6 matmuls/super)
  sc: G[n, z, k] = f @ W_skip[:, z, :]; sc = sum_z a[n, z] * G[n, z, k]
      (TensorE + DVE broadcast mult + strided X-reduce)
"""

import heapq

import numpy as np

import concourse.bass as bass
import concourse.mybir as mybir
import concourse.tile as tile

# ---------------------------------------------------------------- constants
N_NODES = 32768
N_EDGES = 262144
C = 64
N_LM = 16
N_L = 4
N_ELEM = 10
AVG_NUM_NEIGHBORS = 16.0

N_CORES = 8
G_NODES = 16          # nodes per group
G_EDGES = 128         # edge-slot capacity per group (one K=128 matmul)
GROUPS_PER_CORE = 264
TOTAL_GROUPS = N_CORES * GROUPS_PER_CORE
SUPERS = GROUPS_PER_CORE // 8        # 33 super-tiles (8 groups each)
SLOTS = GROUPS_PER_CORE * G_EDGES    # 33792 edge slots per core
NCAP = GROUPS_PER_CORE * G_NODES     # 4224 node slots per core

LM0 = [0, 1, 4, 9]   # first lm index of each l block
ML = [1, 3, 5, 7]    # multiplicity of each l block
F32 = mybir.dt.float32
F32R = mybir.dt.float32r

# final-linear matmul splits: (l, lm_start, lm_stop) per [64,1024] psum half
FINAL_E1 = [(0, 0, 1), (1, 1, 4), (2, 4, 8)]
FINAL_E2 = [(2, 8, 9), (3, 9, 12), (3, 12, 16)]


# ------------------------------------------------------- walrus wait limit
def _split_multiwaits(nc):
    """This container's walrus rejects >1 sync-wait per instruction; move
    excess waits onto NOPs inserted just before the owning instruction."""
    for f in nc.m.functions:
        for bb in f.blocks:
            out = []
            for inst in bb.instructions:
                si = getattr(inst, "sync_info", None)
                waits = list(si.on_wait) if (si is not None and si.on_wait) else []
                if len(waits) > 1:
                    for i, w in enumerate(waits[:-1]):
                        out.append(mybir.InstNoOp(
                            name=f"{inst.name}-wsplit{i}",
                            engine=inst.engine,
                            sync_info=mybir.SyncInfo(on_wait=[w], on_update=[]),
                            bass_nofuse=True,
                        ))
                    si.on_wait = waits[-1:]
                out.append(inst)
            bb.instructions = out


# ------------------------------------------------------------ host sharding
def assign_groups(receiver):
    """Bin-pack nodes into TOTAL_GROUPS groups: <=G_NODES nodes and
    <=G_EDGES total in-degree per group. Returns (group_of, node_local)."""
    deg = np.bincount(receiver, minlength=N_NODES)
    assert deg.max() <= G_EDGES, "node degree exceeds group capacity"
    order = np.argsort(-deg, kind="stable")
    heap = [(0, g) for g in range(TOTAL_GROUPS)]
    heapq.heapify(heap)
    g_edges = np.zeros(TOTAL_GROUPS, np.int64)
    g_nodes = np.zeros(TOTAL_GROUPS, np.int64)
    group_of = np.empty(N_NODES, np.int64)
    node_local = np.empty(N_NODES, np.int64)
    for n in order:
        d = int(deg[n])
        stash = []
        while True:
            e, g = heapq.heappop(heap)
            if g_nodes[g] < G_NODES and g_edges[g] + d <= G_EDGES:
                break
            if g_nodes[g] < G_NODES:
                stash.append((e, g))
        for item in stash:
            heapq.heappush(heap, item)
        group_of[n] = g
        node_local[n] = g_nodes[g]
        g_nodes[g] += 1
        g_edges[g] += d
        if g_nodes[g] < G_NODES:
            heapq.heappush(heap, (int(g_edges[g]), g))
    assert g_edges.max() <= G_EDGES
    return group_of, node_local


def shard_inputs(node_attrs, node_feats, edge_attrs, edge_feats, edge_index):
    """Build the per-core packed arrays. Returns (in_maps, node_ids)."""
    node_attrs = np.asarray(node_attrs, np.float32)
    node_feats = np.asarray(node_feats, np.float32)
    edge_attrs = np.asarray(edge_attrs, np.float32)
    edge_feats = np.asarray(edge_feats, np.float32)
    edge_index = np.asarray(edge_index)
    sender, receiver = edge_index[0].astype(np.int64), edge_index[1].astype(np.int64)

    group_of, node_local = assign_groups(receiver)

    # ---- edge -> slot assignment (slot = local_group*128 + fill position)
    ge = group_of[receiver]                       # group of each edge
    order_e = np.argsort(ge, kind="stable")
    ge_sorted = ge[order_e]
    counts = np.bincount(ge, minlength=TOTAL_GROUPS)
    starts = np.concatenate([[0], np.cumsum(counts)[:-1]])
    pos = np.arange(N_EDGES) - starts[ge_sorted]  # position within group
    core_e = ge_sorted // GROUPS_PER_CORE
    lg_e = ge_sorted % GROUPS_PER_CORE
    slot_e = lg_e * G_EDGES + pos                 # slot within core

    # ---- node -> position within core
    core_n = group_of // GROUPS_PER_CORE
    qpos = (group_of % GROUPS_PER_CORE) * G_NODES + node_local

    in_maps = []
    node_ids = np.full((N_CORES, NCAP), -1, np.int64)
    for core in range(N_CORES):
        me = core_e == core
        eids = order_e[me]
        slots = slot_e[me]

        R = np.zeros((SLOTS, N_L * C), np.float32)
        Y = np.zeros((SLOTS, N_LM), np.float32)
        FT = np.zeros((SLOTS, C), np.float32)
        RV = np.zeros(SLOTS, np.float32)
        R[slots] = edge_feats[eids]
        Y[slots] = edge_attrs[eids]
        FT[slots] = node_feats[sender[eids]]
        RV[slots] = node_local[receiver[eids]].astype(np.float32)

        # DMA layouts
        r_host = (R.reshape(SUPERS, 8, 128, 256).transpose(0, 2, 1, 3)
                  .reshape(SUPERS, 128, 2048).copy())
        ft_host = (FT.reshape(SUPERS, 8, 128, C).transpose(0, 3, 1, 2)
                   .reshape(SUPERS, C, 1024).copy())
        y_host = (Y.reshape(SUPERS, 8, 128, N_LM).transpose(2, 0, 1, 3)
                  .reshape(128, SUPERS * 128).copy())
        rv_host = (RV.reshape(SUPERS, 8, 128).transpose(2, 0, 1)
                   .reshape(128, SUPERS * 8).copy())

        mn = core_n == core
        nids = np.nonzero(mn)[0]
        node_ids[core, qpos[nids]] = nids
        NF = np.zeros((NCAP, C), np.float32)
        NA = np.zeros((NCAP, N_ELEM), np.float32)
        NF[qpos[nids]] = node_feats[nids]
        NA[qpos[nids]] = node_attrs[nids]
        nft_host = NF.T.copy()                                   # [64, 4224]
        at_host = (NA.reshape(SUPERS, 128, N_ELEM).transpose(1, 0, 2)
                   .reshape(128, SUPERS * N_ELEM).copy())

        in_maps.append({
            "r": r_host, "ft": ft_host, "y": y_host, "rv": rv_host,
            "nft": nft_host, "at": at_host,
        })

    return in_maps, node_ids


def weight_maps(W_up, W_linear, W_skip):
    W_up = np.asarray(W_up, np.float32)
    W_linear = np.asarray(W_linear, np.float32)
    W_skip = np.asarray(W_skip, np.float32)
    wup_host = (W_up / np.sqrt(C)).astype(np.float32)              # [64, 64]
    wl_host = (W_linear / (np.sqrt(C) * AVG_NUM_NEIGHBORS)          # [64, 256]
               ).transpose(1, 0, 2).reshape(C, N_L * C).copy()
    ws_host = (W_skip / np.sqrt(C * N_ELEM)).reshape(C, N_ELEM * C).copy()
    return {"wup": wup_host.astype(np.float32),
            "wl": wl_host.astype(np.float32),
            "ws": ws_host.astype(np.float32)}


# ------------------------------------------------------------ device kernel
def build_nc(n_sup=SUPERS, loop_iters=1):
    nc = bass.Bass("TRN2", target_bir_lowering=False)
    r_d = nc.dram_tensor("r", [n_sup, 128, 2048], F32, kind="ExternalInput")
    ft_d = nc.dram_tensor("ft", [n_sup, C, 1024], F32R, kind="ExternalInput")
    y_d = nc.dram_tensor("y", [128, n_sup * 128], F32, kind="ExternalInput")
    rv_d = nc.dram_tensor("rv", [128, n_sup * 8], F32, kind="ExternalInput")
    nft_d = nc.dram_tensor("nft", [C, n_sup * 128], F32R, kind="ExternalInput")
    at_d = nc.dram_tensor("at", [128, n_sup * N_ELEM], F32, kind="ExternalInput")
    wup_d = nc.dram_tensor("wup", [C, C], F32R, kind="ExternalInput")
    wl_d = nc.dram_tensor("wl", [C, N_L * C], F32R, kind="ExternalInput")
    ws_d = nc.dram_tensor("ws", [C, N_ELEM * C], F32R, kind="ExternalInput")
    out_d = nc.dram_tensor("outp", [n_sup, C, 2048], F32, kind="ExternalOutput")
    sc_d = nc.dram_tensor("scp", [128, n_sup * C], F32, kind="ExternalOutput")

    with tile.TileContext(nc) as tc:
        with (
            tc.tile_pool(name="const", bufs=1) as cp,
            tc.tile_pool(name="work", bufs=2) as wp,
            tc.tile_pool(name="wide", bufs=2) as wdp,
            tc.tile_pool(name="ps_xs", bufs=1, space="PSUM") as ps_xs,
            tc.tile_pool(name="ps_g", bufs=1, space="PSUM") as ps_g,
            tc.tile_pool(name="ps_a", bufs=2, space="PSUM") as ps_a,
            tc.tile_pool(name="ps_o", bufs=1, space="PSUM") as ps_o,
        ):
            # ---- whole-kernel preloads
            wup_sb = cp.tile([C, C], F32R)
            wl_sb = cp.tile([C, N_L, C], F32R)
            ws_sb = cp.tile([C, N_ELEM * C], F32R)
            y_all = cp.tile([128, n_sup * 128], F32)
            rv_all = cp.tile([128, n_sup * 8], F32)
            nft_all = cp.tile([C, n_sup * 128], F32R)
            at_all = cp.tile([128, n_sup, N_ELEM], F32)
            sc_buf = cp.tile([128, n_sup * C], F32)
            nc.sync.dma_start(out=wup_sb[:], in_=wup_d[:])
            nc.sync.dma_start(out=wl_sb[:].rearrange("p l c -> p (l c)"), in_=wl_d[:])
            nc.sync.dma_start(out=ws_sb[:], in_=ws_d[:])
            nc.sync.dma_start(out=y_all[:], in_=y_d[:])
            nc.sync.dma_start(out=rv_all[:], in_=rv_d[:])
            nc.sync.dma_start(out=nft_all[:], in_=nft_d[:])
            nc.sync.dma_start(out=at_all[:].rearrange("p s z -> p (s z)"), in_=at_d[:])
            iota_i = cp.tile([128, G_NODES], mybir.dt.int32)
            iota_f = cp.tile([128, G_NODES], F32)
            nc.gpsimd.iota(iota_i[:], pattern=[[1, G_NODES]], base=0,
                           channel_multiplier=0)
            nc.vector.tensor_copy(iota_f[:], iota_i[:])

            def super_body(sup):
                # ---------------- sc (skip tensor product) for this node tile
                G_ps = ps_g.tile([128, N_ELEM, C], F32, tag="g")
                nc.tensor.matmul(out=G_ps[:].rearrange("p z k -> p (z k)")[:, 0:512],
                                 lhsT=nft_all[:, sup * 128:(sup + 1) * 128],
                                 rhs=ws_sb[:, 0:512], start=True, stop=True)
                nc.tensor.matmul(out=G_ps[:].rearrange("p z k -> p (z k)")[:, 512:640],
                                 lhsT=nft_all[:, sup * 128:(sup + 1) * 128],
                                 rhs=ws_sb[:, 512:640], start=True, stop=True)
                tmp_sc = wp.tile([128, N_ELEM, C], F32, tag="tmpsc")
                nc.vector.tensor_tensor(
                    out=tmp_sc[:], in0=G_ps[:],
                    in1=at_all[:, sup, :, None].to_broadcast([128, N_ELEM, C]),
                    op=mybir.AluOpType.mult)
                nc.vector.tensor_reduce(
                    out=sc_buf[:, sup * C:(sup + 1) * C],
                    in_=tmp_sc[:].rearrange("p z k -> p k z"),
                    axis=mybir.AxisListType.X, op=mybir.AluOpType.add)

                # ---------------- edge-side loads
                r_t = wdp.tile([128, 8, N_L, C], F32, tag="r")
                ft_t = wp.tile([C, 8, 128], F32R, tag="ft")
                nc.sync.dma_start(out=r_t[:].rearrange("p a b c -> p (a b c)"),
                                  in_=r_d[sup])
                nc.sync.dma_start(out=ft_t[:].rearrange("p a b -> p (a b)"),
                                  in_=ft_d[sup])

                # ---------------- xs = F @ W_up
                xs_ps = ps_xs.tile([128, 8, C], F32, tag="xs")
                for j in range(8):
                    nc.tensor.matmul(out=xs_ps[:, j, :], lhsT=ft_t[:, j, :],
                                     rhs=wup_sb[:], start=True, stop=True)
                xs_sb = wp.tile([128, 8, C], F32, tag="xs_sb")
                nc.scalar.copy(out=xs_sb[:], in_=xs_ps[:])

                # ---------------- S, SY, z
                s_t = wp.tile([128, 8, G_NODES], F32, tag="s")
                nc.vector.tensor_tensor(
                    out=s_t[:],
                    in0=iota_f[:, None, :].to_broadcast([128, 8, G_NODES]),
                    in1=rv_all[:, sup * 8:(sup + 1) * 8, None]
                        .to_broadcast([128, 8, G_NODES]),
                    op=mybir.AluOpType.is_equal)
                sy_t = wdp.tile([128, 8, N_LM, G_NODES], F32R, tag="sy")
                y_v = y_all[:].rearrange("p (s j lm) -> p s j lm", s=n_sup, j=8)
                nc.vector.tensor_tensor(
                    out=sy_t[:],
                    in0=y_v[:, sup, :, :, None].to_broadcast([128, 8, N_LM, G_NODES]),
                    in1=s_t[:, :, None, :].to_broadcast([128, 8, N_LM, G_NODES]),
                    op=mybir.AluOpType.mult)
                z_t = wdp.tile([128, 8, N_L, C], F32R, tag="z")
                nc.gpsimd.tensor_tensor(
                    out=z_t[:], in0=r_t[:],
                    in1=xs_sb[:, :, None, :].to_broadcast([128, 8, N_L, C]),
                    op=mybir.AluOpType.mult)

                # ---------------- scatter matmuls (per group pair) + copyback
                a_sb = wp.tile([C, 8, N_LM, G_NODES], F32R, tag="a_sb")
                for jp in range(4):
                    a_ps = ps_a.tile([C, 2, N_LM, G_NODES], F32, tag="a_ps")
                    for jj in range(2):
                        j = jp * 2 + jj
                        for l in range(4):
                            nc.tensor.matmul(
                                out=a_ps[:, jj, LM0[l]:LM0[l] + ML[l], :],
                                lhsT=z_t[:, j, l, :],
                                rhs=sy_t[:, j, LM0[l]:LM0[l] + ML[l], :],
                                start=True, stop=True)
                    nc.scalar.copy(out=a_sb[:, jp * 2:jp * 2 + 2, :, :], in_=a_ps[:])

                # ---------------- final per-l linear
                ob = wp.tile([C, 2048], F32, tag="ob")
                for half, plan in ((0, FINAL_E1), (1, FINAL_E2)):
                    o_ps = ps_o.tile([C, 1024], F32, tag="o_ps")
                    base = 0 if half == 0 else 8
                    for (l, lma, lmb) in plan:
                        nc.tensor.matmul(
                            out=o_ps[:, (lma - base) * 128:(lmb - base) * 128],
                            lhsT=wl_sb[:, l, :],
                            rhs=a_sb[:, :, lma:lmb, :].rearrange(
                                "p j m n -> p m j n"),
                            start=True, stop=True)
                    nc.scalar.copy(out=ob[:, half * 1024:(half + 1) * 1024],
                                   in_=o_ps[:])
                nc.sync.dma_start(out=out_d[sup], in_=ob[:])

            if loop_iters == 1:
                for sup in range(n_sup):
                    super_body(sup)
                nc.sync.dma_start(out=sc_d[:], in_=sc_buf[:])
            else:
                with tc.For_i(0, loop_iters, 1):
                    for sup in range(n_sup):
                        super_body(sup)
                    nc.sync.dma_start(out=sc_d[:], in_=sc_buf[:])

    _split_multiwaits(nc)
    return nc


# ------------------------------------------------------------ output gather
def assemble(results, node_ids, n_sup=SUPERS):
    out = np.zeros((N_NODES, N_LM, C), np.float32)
    sc = np.zeros((N_NODES, C), np.float32)
    for core in range(N_CORES):
        o = results[core]["outp"]          # [n_sup, 64, 2048]
        arr = (o.reshape(n_sup, C, N_LM, 8, G_NODES)
               .transpose(0, 3, 4, 2, 1).reshape(n_sup * 128, N_LM, C))
        s = (results[core]["scp"].reshape(128, n_sup, C)
             .transpose(1, 0, 2).reshape(n_sup * 128, C))
        ids = node_ids[core][:n_sup * 128]
        valid = ids >= 0
        out[ids[valid]] = arr[valid]
        sc[ids[valid]] = s[valid]
    return out, sc


# ------------------------------------------------------------ entry point
def kernel(node_attrs, node_feats, edge_attrs, edge_feats, edge_index,
           W_up, W_linear, W_skip):
    from concourse.bass_utils import run_bass_kernel_spmd

    in_maps, node_ids = shard_inputs(node_attrs, node_feats, edge_attrs,
                                     edge_feats, edge_index)
    wmap = weight_maps(W_up, W_linear, W_skip)
    for m in in_maps:
        m.update(wmap)
    nc = build_nc()
    res = run_bass_kernel_spmd(nc, in_maps, core_ids=list(range(N_CORES)))
    out, sc = assemble(res.results, node_ids)
    return out, sc


# revision 7
# speedup vs baseline: 2.0986x; 2.0986x over previous
"""Trainium2 Bass kernel for nn_InvariantResidualInteraction (8-core SPMD).

Strategy (graph/data parallel, per sharding hint):
  - Partition nodes across the 8 cores; each edge is owned by its receiver's
    core. Host-side "halo exchange": sender features are gathered per edge at
    sharding time and uploaded in the layout the kernel wants.
  - Nodes are bin-packed by in-degree into groups of <=16 nodes whose total
    in-degree is <=128, so each group's messages reduce with one K=128
    scatter-matmul (selection matrix built on-device from receiver indices).
  - Weights are replicated; scalar normalizations are folded into the weights.

Device pipeline per super-tile (8 groups = 1024 edge slots = 128 nodes):
  xs = F_gathered @ W_up            (TensorE, per 128-slot tile)
  S[e, n]   = (iota_n == recv_local[e])            (DVE iota+is_equal)
  SY[e, lm, n] = Y[e, lm] * S[e, n]                (DVE broadcast mult)
  z[e, l, c]   = R[e, l, c] * xs[e, c]             (DVE broadcast mult)
  A[c, lm, n]  = sum_e z[e, l(lm), c] * SY[e, lm, n]   (TensorE, 4 matmuls/group)
  out[k, lm, n] = sum_c W_l[c, k] * A[c, lm, n]        (TensorE, 6 matmuls/super)
  sc: G[n, z, k] = f @ W_skip[:, z, :]; sc = sum_z a[n, z] * G[n, z, k]
      (TensorE + DVE broadcast mult + strided X-reduce)
"""

import heapq

import numpy as np

import concourse.bass as bass
import concourse.mybir as mybir
import concourse.tile as tile

# ---------------------------------------------------------------- constants
N_NODES = 32768
N_EDGES = 262144
C = 64
N_LM = 16
N_L = 4
N_ELEM = 10
AVG_NUM_NEIGHBORS = 16.0

N_CORES = 8
G_NODES = 16          # nodes per group
G_EDGES = 128         # edge-slot capacity per group (one K=128 matmul)
GROUPS_PER_CORE = 264
TOTAL_GROUPS = N_CORES * GROUPS_PER_CORE
SUPERS = GROUPS_PER_CORE // 8        # 33 super-tiles (8 groups each)
SLOTS = GROUPS_PER_CORE * G_EDGES    # 33792 edge slots per core
NCAP = GROUPS_PER_CORE * G_NODES     # 4224 node slots per core

LM0 = [0, 1, 4, 9]   # first lm index of each l block
ML = [1, 3, 5, 7]    # multiplicity of each l block
F32 = mybir.dt.float32
F32R = mybir.dt.float32r

# final-linear matmul splits: (l, lm_start, lm_stop) per [64,1024] psum half
FINAL_E1 = [(0, 0, 1), (1, 1, 4), (2, 4, 8)]
FINAL_E2 = [(2, 8, 9), (3, 9, 12), (3, 12, 16)]


# ------------------------------------------------------- walrus wait limit
def _split_multiwaits(nc):
    """This container's walrus rejects >1 sync-wait per instruction; move
    excess waits onto NOPs inserted just before the owning instruction."""
    for f in nc.m.functions:
        for bb in f.blocks:
            out = []
            for inst in bb.instructions:
                si = getattr(inst, "sync_info", None)
                waits = list(si.on_wait) if (si is not None and si.on_wait) else []
                if len(waits) > 1:
                    for i, w in enumerate(waits[:-1]):
                        out.append(mybir.InstNoOp(
                            name=f"{inst.name}-wsplit{i}",
                            engine=inst.engine,
                            sync_info=mybir.SyncInfo(on_wait=[w], on_update=[]),
                            bass_nofuse=True,
                        ))
                    si.on_wait = waits[-1:]
                out.append(inst)
            bb.instructions = out


# ------------------------------------------------------------ host sharding
def assign_groups(receiver):
    """Bin-pack nodes into TOTAL_GROUPS groups: <=G_NODES nodes and
    <=G_EDGES total in-degree per group. Returns (group_of, node_local)."""
    deg = np.bincount(receiver, minlength=N_NODES)
    assert deg.max() <= G_EDGES, "node degree exceeds group capacity"
    order = np.argsort(-deg, kind="stable")
    heap = [(0, g) for g in range(TOTAL_GROUPS)]
    heapq.heapify(heap)
    g_edges = np.zeros(TOTAL_GROUPS, np.int64)
    g_nodes = np.zeros(TOTAL_GROUPS, np.int64)
    group_of = np.empty(N_NODES, np.int64)
    node_local = np.empty(N_NODES, np.int64)
    for n in order:
        d = int(deg[n])
        stash = []
        while True:
            e, g = heapq.heappop(heap)
            if g_nodes[g] < G_NODES and g_edges[g] + d <= G_EDGES:
                break
            if g_nodes[g] < G_NODES:
                stash.append((e, g))
        for item in stash:
            heapq.heappush(heap, item)
        group_of[n] = g
        node_local[n] = g_nodes[g]
        g_nodes[g] += 1
        g_edges[g] += d
        if g_nodes[g] < G_NODES:
            heapq.heappush(heap, (int(g_edges[g]), g))
    assert g_edges.max() <= G_EDGES
    return group_of, node_local


def shard_inputs(node_attrs, node_feats, edge_attrs, edge_feats, edge_index):
    """Build the per-core packed arrays. Returns (in_maps, node_ids)."""
    node_attrs = np.asarray(node_attrs, np.float32)
    node_feats = np.asarray(node_feats, np.float32)
    edge_attrs = np.asarray(edge_attrs, np.float32)
    edge_feats = np.asarray(edge_feats, np.float32)
    edge_index = np.asarray(edge_index)
    sender, receiver = edge_index[0].astype(np.int64), edge_index[1].astype(np.int64)

    group_of, node_local = assign_groups(receiver)

    # ---- edge -> slot assignment (slot = local_group*128 + fill position)
    ge = group_of[receiver]                       # group of each edge
    order_e = np.argsort(ge, kind="stable")
    ge_sorted = ge[order_e]
    counts = np.bincount(ge, minlength=TOTAL_GROUPS)
    starts = np.concatenate([[0], np.cumsum(counts)[:-1]])
    pos = np.arange(N_EDGES) - starts[ge_sorted]  # position within group
    core_e = ge_sorted // GROUPS_PER_CORE
    lg_e = ge_sorted % GROUPS_PER_CORE
    slot_e = lg_e * G_EDGES + pos                 # slot within core

    # ---- node -> position within core
    core_n = group_of // GROUPS_PER_CORE
    qpos = (group_of % GROUPS_PER_CORE) * G_NODES + node_local

    in_maps = []
    node_ids = np.full((N_CORES, NCAP), -1, np.int64)
    for core in range(N_CORES):
        me = core_e == core
        eids = order_e[me]
        slots = slot_e[me]

        R = np.zeros((SLOTS, N_L * C), np.float32)
        Y = np.zeros((SLOTS, N_LM), np.float32)
        FT = np.zeros((SLOTS, C), np.float32)
        RV = np.zeros(SLOTS, np.float32)
        R[slots] = edge_feats[eids]
        Y[slots] = edge_attrs[eids]
        FT[slots] = node_feats[sender[eids]]
        RV[slots] = node_local[receiver[eids]].astype(np.float32)

        # DMA layouts
        r_host = (R.reshape(SUPERS, 8, 128, 256).transpose(0, 2, 1, 3)
                  .reshape(SUPERS, 128, 2048).copy())
        ft_host = (FT.reshape(SUPERS, 8, 128, C).transpose(0, 3, 1, 2)
                   .reshape(SUPERS, C, 1024).copy())
        y_host = (Y.reshape(SUPERS, 8, 128, N_LM).transpose(2, 0, 1, 3)
                  .reshape(128, SUPERS * 128).copy())
        rv_host = (RV.reshape(SUPERS, 8, 128).transpose(2, 0, 1)
                   .reshape(128, SUPERS * 8).copy())

        mn = core_n == core
        nids = np.nonzero(mn)[0]
        node_ids[core, qpos[nids]] = nids
        NF = np.zeros((NCAP, C), np.float32)
        NA = np.zeros((NCAP, N_ELEM), np.float32)
        NF[qpos[nids]] = node_feats[nids]
        NA[qpos[nids]] = node_attrs[nids]
        nft_host = NF.T.copy()                                   # [64, 4224]
        at_host = (NA.reshape(SUPERS, 128, N_ELEM).transpose(1, 0, 2)
                   .reshape(128, SUPERS * N_ELEM).copy())

        in_maps.append({
            "r": r_host, "ft": ft_host, "y": y_host, "rv": rv_host,
            "nft": nft_host, "at": at_host,
        })

    return in_maps, node_ids


def weight_maps(W_up, W_linear, W_skip):
    W_up = np.asarray(W_up, np.float32)
    W_linear = np.asarray(W_linear, np.float32)
    W_skip = np.asarray(W_skip, np.float32)
    wup_host = (W_up / np.sqrt(C)).astype(np.float32)              # [64, 64]
    wl_host = (W_linear / (np.sqrt(C) * AVG_NUM_NEIGHBORS)          # [64, 256]
               ).transpose(1, 0, 2).reshape(C, N_L * C).copy()
    ws_host = (W_skip / np.sqrt(C * N_ELEM)).reshape(C, N_ELEM * C).copy()
    return {"wup": wup_host.astype(np.float32),
            "wl": wl_host.astype(np.float32),
            "ws": ws_host.astype(np.float32)}


# ------------------------------------------------------------ device kernel
def build_nc(n_sup=SUPERS, loop_iters=1):
    nc = bass.Bass("TRN2", target_bir_lowering=False)
    r_d = nc.dram_tensor("r", [n_sup, 128, 2048], F32, kind="ExternalInput")
    ft_d = nc.dram_tensor("ft", [n_sup, C, 1024], F32R, kind="ExternalInput")
    y_d = nc.dram_tensor("y", [128, n_sup * 128], F32, kind="ExternalInput")
    rv_d = nc.dram_tensor("rv", [128, n_sup * 8], F32, kind="ExternalInput")
    nft_d = nc.dram_tensor("nft", [C, n_sup * 128], F32R, kind="ExternalInput")
    at_d = nc.dram_tensor("at", [128, n_sup * N_ELEM], F32, kind="ExternalInput")
    wup_d = nc.dram_tensor("wup", [C, C], F32R, kind="ExternalInput")
    wl_d = nc.dram_tensor("wl", [C, N_L * C], F32R, kind="ExternalInput")
    ws_d = nc.dram_tensor("ws", [C, N_ELEM * C], F32R, kind="ExternalInput")
    out_d = nc.dram_tensor("outp", [n_sup, C, 2048], F32, kind="ExternalOutput")
    sc_d = nc.dram_tensor("scp", [128, n_sup * C], F32, kind="ExternalOutput")

    with tile.TileContext(nc) as tc:
        with (
            tc.tile_pool(name="const", bufs=1) as cp,
            tc.tile_pool(name="work", bufs=3) as wp,
            tc.tile_pool(name="wide", bufs=3) as wdp,
            tc.tile_pool(name="ps_xs", bufs=2, space="PSUM") as ps_xs,
            tc.tile_pool(name="ps_g", bufs=1, space="PSUM") as ps_g,
            tc.tile_pool(name="ps_a", bufs=2, space="PSUM") as ps_a,
            tc.tile_pool(name="ps_o", bufs=1, space="PSUM") as ps_o,
        ):
            # ---- whole-kernel preloads
            wup_sb = cp.tile([C, C], F32R)
            wl_sb = cp.tile([C, N_L, C], F32R)
            ws_sb = cp.tile([C, N_ELEM * C], F32R)
            y_all = cp.tile([128, n_sup * 128], F32)
            rv_all = cp.tile([128, n_sup * 8], F32)
            nft_all = cp.tile([C, n_sup * 128], F32R)
            at_all = cp.tile([128, n_sup, N_ELEM], F32)
            sc_buf = cp.tile([128, n_sup * C], F32)
            nc.sync.dma_start(out=wup_sb[:], in_=wup_d[:])
            nc.sync.dma_start(out=wl_sb[:].rearrange("p l c -> p (l c)"), in_=wl_d[:])
            nc.sync.dma_start(out=ws_sb[:], in_=ws_d[:])
            nc.sync.dma_start(out=y_all[:], in_=y_d[:])
            nc.sync.dma_start(out=rv_all[:], in_=rv_d[:])
            nc.sync.dma_start(out=nft_all[:], in_=nft_d[:])
            nc.sync.dma_start(out=at_all[:].rearrange("p s z -> p (s z)"), in_=at_d[:])
            iota_i = cp.tile([128, G_NODES], mybir.dt.int32)
            iota_f = cp.tile([128, G_NODES], F32)
            nc.gpsimd.iota(iota_i[:], pattern=[[1, G_NODES]], base=0,
                           channel_multiplier=0)
            nc.vector.tensor_copy(iota_f[:], iota_i[:])

            def super_body(sup):
                # ---------------- sc (skip tensor product) for this node tile
                G_ps = ps_g.tile([128, N_ELEM, C], F32, tag="g")
                nc.tensor.matmul(out=G_ps[:].rearrange("p z k -> p (z k)")[:, 0:512],
                                 lhsT=nft_all[:, sup * 128:(sup + 1) * 128],
                                 rhs=ws_sb[:, 0:512], start=True, stop=True)
                nc.tensor.matmul(out=G_ps[:].rearrange("p z k -> p (z k)")[:, 512:640],
                                 lhsT=nft_all[:, sup * 128:(sup + 1) * 128],
                                 rhs=ws_sb[:, 512:640], start=True, stop=True)
                tmp_sc = wp.tile([128, N_ELEM, C], F32, tag="tmpsc")
                nc.vector.tensor_tensor(
                    out=tmp_sc[:], in0=G_ps[:],
                    in1=at_all[:, sup, :, None].to_broadcast([128, N_ELEM, C]),
                    op=mybir.AluOpType.mult)
                nc.vector.tensor_reduce(
                    out=sc_buf[:, sup * C:(sup + 1) * C],
                    in_=tmp_sc[:].rearrange("p z k -> p k z"),
                    axis=mybir.AxisListType.X, op=mybir.AluOpType.add)

                # ---------------- edge-side loads
                r_t = wdp.tile([128, 8, N_L, C], F32, tag="r")
                ft_t = wp.tile([C, 8, 128], F32R, tag="ft")
                nc.sync.dma_start(out=r_t[:].rearrange("p a b c -> p (a b c)"),
                                  in_=r_d[sup])
                nc.sync.dma_start(out=ft_t[:].rearrange("p a b -> p (a b)"),
                                  in_=ft_d[sup])

                # ---------------- xs = F @ W_up
                xs_ps = ps_xs.tile([128, 8, C], F32, tag="xs")
                for j in range(8):
                    nc.tensor.matmul(out=xs_ps[:, j, :], lhsT=ft_t[:, j, :],
                                     rhs=wup_sb[:], start=True, stop=True)
                xs_sb = wp.tile([128, 8, C], F32, tag="xs_sb")
                nc.scalar.copy(out=xs_sb[:], in_=xs_ps[:])

                # ---------------- S, SY, z
                s_t = wp.tile([128, 8, G_NODES], F32, tag="s")
                nc.vector.tensor_tensor(
                    out=s_t[:],
                    in0=iota_f[:, None, :].to_broadcast([128, 8, G_NODES]),
                    in1=rv_all[:, sup * 8:(sup + 1) * 8, None]
                        .to_broadcast([128, 8, G_NODES]),
                    op=mybir.AluOpType.is_equal)
                sy_t = wdp.tile([128, 8, N_LM, G_NODES], F32R, tag="sy")
                y_v = y_all[:].rearrange("p (s j lm) -> p s j lm", s=n_sup, j=8)
                nc.vector.tensor_tensor(
                    out=sy_t[:],
                    in0=y_v[:, sup, :, :, None].to_broadcast([128, 8, N_LM, G_NODES]),
                    in1=s_t[:, :, None, :].to_broadcast([128, 8, N_LM, G_NODES]),
                    op=mybir.AluOpType.mult)
                z_t = wdp.tile([128, 8, N_L, C], F32R, tag="z")
                nc.vector.tensor_tensor(
                    out=z_t[:], in0=r_t[:],
                    in1=xs_sb[:, :, None, :].to_broadcast([128, 8, N_L, C]),
                    op=mybir.AluOpType.mult)

                # ---------------- scatter matmuls (per group pair) + copyback
                a_sb = wp.tile([C, 8, N_LM, G_NODES], F32R, tag="a_sb")
                for jp in range(4):
                    a_ps = ps_a.tile([C, 2, N_LM, G_NODES], F32, tag="a_ps")
                    for jj in range(2):
                        j = jp * 2 + jj
                        for l in range(4):
                            nc.tensor.matmul(
                                out=a_ps[:, jj, LM0[l]:LM0[l] + ML[l], :],
                                lhsT=z_t[:, j, l, :],
                                rhs=sy_t[:, j, LM0[l]:LM0[l] + ML[l], :],
                                start=True, stop=True)
                    nc.scalar.copy(out=a_sb[:, jp * 2:jp * 2 + 2, :, :], in_=a_ps[:])

                # ---------------- final per-l linear
                ob = wp.tile([C, 2048], F32, tag="ob")
                for half, plan in ((0, FINAL_E1), (1, FINAL_E2)):
                    o_ps = ps_o.tile([C, 1024], F32, tag="o_ps")
                    base = 0 if half == 0 else 8
                    for (l, lma, lmb) in plan:
                        nc.tensor.matmul(
                            out=o_ps[:, (lma - base) * 128:(lmb - base) * 128],
                            lhsT=wl_sb[:, l, :],
                            rhs=a_sb[:, :, lma:lmb, :].rearrange(
                                "p j m n -> p m j n"),
                            start=True, stop=True)
                    nc.scalar.copy(out=ob[:, half * 1024:(half + 1) * 1024],
                                   in_=o_ps[:])
                nc.sync.dma_start(out=out_d[sup], in_=ob[:])

            if loop_iters == 1:
                for sup in range(n_sup):
                    super_body(sup)
                nc.sync.dma_start(out=sc_d[:], in_=sc_buf[:])
            else:
                with tc.For_i(0, loop_iters, 1):
                    for sup in range(n_sup):
                        super_body(sup)
                    nc.sync.dma_start(out=sc_d[:], in_=sc_buf[:])

    _split_multiwaits(nc)
    return nc


# ------------------------------------------------------------ output gather
def assemble(results, node_ids, n_sup=SUPERS):
    out = np.zeros((N_NODES, N_LM, C), np.float32)
    sc = np.zeros((N_NODES, C), np.float32)
    for core in range(N_CORES):
        o = results[core]["outp"]          # [n_sup, 64, 2048]
        arr = (o.reshape(n_sup, C, N_LM, 8, G_NODES)
               .transpose(0, 3, 4, 2, 1).reshape(n_sup * 128, N_LM, C))
        s = (results[core]["scp"].reshape(128, n_sup, C)
             .transpose(1, 0, 2).reshape(n_sup * 128, C))
        ids = node_ids[core][:n_sup * 128]
        valid = ids >= 0
        out[ids[valid]] = arr[valid]
        sc[ids[valid]] = s[valid]
    return out, sc


# ------------------------------------------------------------ entry point
def kernel(node_attrs, node_feats, edge_attrs, edge_feats, edge_index,
           W_up, W_linear, W_skip):
    from concourse.bass_utils import run_bass_kernel_spmd

    in_maps, node_ids = shard_inputs(node_attrs, node_feats, edge_attrs,
                                     edge_feats, edge_index)
    wmap = weight_maps(W_up, W_linear, W_skip)
    for m in in_maps:
        m.update(wmap)
    nc = build_nc()
    res = run_bass_kernel_spmd(nc, in_maps, core_ids=list(range(N_CORES)))
    out, sc = assemble(res.results, node_ids)
    return out, sc
